# revision 19
# baseline (speedup 1.0000x reference)
"""CycleFC (1-bit weights/activations) Trainium2 kernel.

Computes, for x (B=32, C=384, H=56, W=56), weight (C, C), bias (C,):
    xb = sign(x); wb = sign(weight)
    shifted[b,c,h,w] = xb[b,c,h,w+dx_c]  (0 outside [0,W)), dx_c = (c+3)%7-3
    out = einsum('bchw,oc->bohw', shifted, wb) + bias

Strategy (8 NeuronCores, SPMD):
  - Data-parallel over batch: 4 batches per core; weight/bias replicated.
  - The host packs each core's x slice CHANNEL-MAJOR [C, BL, H, WPAD] with
    each 56-wide row padded to 59 with zeros.  The per-channel horizontal
    shift then folds into the input DMA for free: for a fixed shift dx,
    the shifted data is just the channel's flat [BL*H*WPAD] block read at
    offset +dx -- positions that fall outside [0, W) pick up row padding,
    which is exactly the required zero padding.  Channel-major order makes
    one DMA per shift-group segment cover ALL local batches, so the whole
    input load is 9 large DMAs (plus 9 single-batch ones for batch 0 to
    shorten the pipeline fill) -- few enough to sit entirely inside Tile's
    8-deep in-flight SWDGE window.
  - Channels are processed in a permuted order (grouped by c mod 7 ==
    constant shift) so each shift group is a partition-contiguous,
    channel-stride-7 affine DMA segment.  The weight matrix is permuted
    identically on the host (pure layout transform, no arithmetic), which
    leaves the GEMM result unchanged.
  - Loads are SWDGE (gpsimd) with an inline fp32->bf16 cast (+-1 is exact
    in bf16 and the 384-term accumulation is exact in fp32 PSUM, so the
    result is bit-identical to an fp32 computation).
  - sign() runs on the Scalar engine reading the padded strided view and
    writing a compact [128, H*W] tile, so matmul rhs slices are contiguous.
  - GEMM: out[o, p] = sum_c wbT[c, o] * xb[c, p] on the Tensor engine,
    K = 384 contracted in 3 chunks of 128, k-outer over 7 live PSUM banks
    so the stationary weights are reused across pixel tiles.
  - Bias add fused into the PSUM -> SBUF drain on the Vector engine, into
    full-plane tiles so stores have 12.5 KB contiguous runs per partition;
    stores ride the Sync engine's HWDGE ring, separate from load rings.
"""

import numpy as np

import concourse.bass as bass
import concourse.tile as tile
from concourse import bacc, mybir
from concourse.bass_utils import run_bass_kernel_spmd

# Problem constants (hardcoded per spec)
B, C, H, W = 32, 384, 56, 56
PLANE = H * W              # 3136 (unpadded output plane)
NCORES = 8
BL = B // NCORES           # 4 batches per core
KS = 7                     # cyclic shift period (kernel_size 7)
NK = C // 128              # 3 contraction chunks
NM = C // 128              # 3 output-channel chunks
ROWS_PER_TILE = 8
NTILE = ROWS_PER_TILE * W  # 448 pixels per PSUM tile
NN = H // ROWS_PER_TILE    # 7 pixel tiles per (b, m)
WPAD = 59                  # row pitch: 56 data + 3 zero cols (>= max |dx|)
PLANE_P = H * WPAD         # 3304 (padded input plane)
CBLK = BL * PLANE_P        # one channel's contiguous block in the input
BACK_PAD = 6 * CBLK + PLANE_P + 64  # slack for segment AP over-claim
NX_ELEMS = C * CBLK + BACK_PAD
NOUT_ELEMS = BL * C * PLANE

# Shift-group segments of the permuted channel order.  perm = channels
# grouped by r = c mod 7 (r ascending, then c ascending within the group).
# Each segment is a partition-contiguous run inside one 128-channel chunk:
# (chunk, part_start, nseg, c_first, dx) with original channels
# c_first + 7*i for i in [0, nseg).
SEGMENTS = [
    (0, 0, 55, 0, 0),
    (0, 55, 55, 1, 1),
    (0, 110, 18, 2, 2),
    (1, 0, 37, 128, 2),    # r=2 continued: 2 + 7*18
    (1, 37, 55, 3, 3),
    (1, 92, 36, 4, -3),
    (2, 0, 19, 256, -3),   # r=4 continued: 4 + 7*36
    (2, 19, 55, 5, -2),
    (2, 74, 54, 6, -1),
]

PERM = np.concatenate([np.arange(r, C, KS) for r in range(KS)])

_COMPILED = None


def _build_program():
    """Trace + compile the single-core Bass program (same on all 8 cores)."""
    nc = bacc.Bacc(
        "TRN2",
        target_bir_lowering=False,
        debug=False,
        num_devices=NCORES,
    )
    x_d = nc.dram_tensor("x", [NX_ELEMS], mybir.dt.float32, kind="ExternalInput")
    w_d = nc.dram_tensor("wt", [C, C], mybir.dt.float32, kind="ExternalInput")
    b_d = nc.dram_tensor("bias", [C], mybir.dt.float32, kind="ExternalInput")
    o_d = nc.dram_tensor("out", [NOUT_ELEMS], mybir.dt.float32, kind="ExternalOutput")

    x_ap = x_d.ap()
    o_ap = o_d.ap()

    segs_by_chunk = [[s[1:] for s in SEGMENTS if s[0] == k] for k in range(NK)]

    def seg_src(c_first, dx, nseg, b0, nb):
        """DRAM AP [nseg, nb*PLANE_P]: channels c_first+7i, batches b0..b0+nb,
        flat-shifted by dx (channel-major layout)."""
        base = c_first * CBLK + b0 * PLANE_P + dx
        return (
            x_ap[base : base + nseg * KS * CBLK]
            .rearrange("(p q) -> p q", q=KS * CBLK)[:, : nb * PLANE_P]
        )

    with tile.TileContext(nc) as tc:
        with (
            tc.tile_pool(name="const", bufs=1) as cpool,
            tc.tile_pool(name="xbr", bufs=1) as xbr_pool,
            tc.tile_pool(name="xbc", bufs=9) as xbc_pool,
            tc.tile_pool(name="psum", bufs=8, space="PSUM") as psum_pool,
            tc.tile_pool(name="outs", bufs=3) as out_pool,
        ):
            # Weights/bias first on the SWDGE ring so they complete before
            # the big x loads contend for the SDMA engines.
            w_bf = []
            wraws = []
            for k in range(NK):
                wraw = cpool.tile([128, C], mybir.dt.float32, tag=f"wraw{k}")
                nc.gpsimd.dma_start(wraw[:], w_d.ap()[128 * k : 128 * (k + 1), :])
                wraws.append(wraw)
            bias_t = []
            for m in range(NM):
                bt = cpool.tile([128, 1], mybir.dt.float32, tag=f"bias{m}")
                nc.gpsimd.dma_start(bt[:], b_d.ap()[128 * m : 128 * (m + 1)].unsqueeze(1))
                bias_t.append(bt)
            for k in range(NK):
                wb = cpool.tile([128, C], mybir.dt.bfloat16, tag=f"wb{k}")
                nc.scalar.sign(wb[:], wraws[k][:])
                w_bf.append(wb)

            # x loads, SWDGE with inline fp32->bf16 cast.  Batch 0 rides its
            # own small DMAs (short pipeline fill); batches 1..3 come in one
            # DMA per segment.  18 DMAs total for the whole input.
            xbr0 = []   # per chunk: [128, PLANE_P] (batch 0)
            xbr123 = [] # per chunk: [128, 3*PLANE_P] (batches 1-3)
            for k in range(NK):
                t0 = xbr_pool.tile(
                    [128, PLANE_P], mybir.dt.bfloat16, tag=f"xbr0_{k}"
                )
                for (part_start, nseg, c_first, dx) in segs_by_chunk[k]:
                    nc.gpsimd.dma_start(
                        t0[part_start : part_start + nseg, :],
                        seg_src(c_first, dx, nseg, 0, 1),
                    )
                xbr0.append(t0)
            for k in range(NK):
                t1 = xbr_pool.tile(
                    [128, (BL - 1) * PLANE_P], mybir.dt.bfloat16, tag=f"xbr123_{k}"
                )
                for (part_start, nseg, c_first, dx) in segs_by_chunk[k]:
                    nc.gpsimd.dma_start(
                        t1[part_start : part_start + nseg, :],
                        seg_src(c_first, dx, nseg, 1, BL - 1),
                    )
                xbr123.append(t1)

            # Sign is split at an n-tile boundary (rows 0-23 / 24-55) so the
            # first matmuls of each k-row unblock after half the binarize.
            HSPLIT = 3 * ROWS_PER_TILE  # 24 rows

            for b in range(BL):
                xbcs = []
                for k in range(NK):
                    if b == 0:
                        src = xbr0[k][:]
                    else:
                        src = xbr123[k][:, (b - 1) * PLANE_P : b * PLANE_P]
                    srcv = src.rearrange("p (h w) -> p h w", w=WPAD)[:, :, :W]
                    # Binarize + drop the pad columns: strided read of the
                    # [H, :W] view, contiguous [128, H*W] write.
                    xbc = xbc_pool.tile(
                        [128, PLANE], mybir.dt.bfloat16, tag="xbc", name=f"xbc{b}_{k}"
                    )
                    dstv = xbc[:].rearrange("p (h w) -> p h w", w=W)
                    nc.scalar.sign(dstv[:, :HSPLIT, :], srcv[:, :HSPLIT, :])
                    nc.scalar.sign(dstv[:, HSPLIT:, :], srcv[:, HSPLIT:, :])
                    xbcs.append(xbc)

                for m in range(NM):
                    pss = [
                        psum_pool.tile(
                            [128, NTILE], mybir.dt.float32, tag="ps", name=f"ps{b}_{m}_{n}"
                        )
                        for n in range(NN)
                    ]
                    # k-outer: the stationary weight chunk is reused across
                    # the 7 pixel tiles; PSUM accumulates across k.
                    for k in range(NK):
                        for n in range(NN):
                            nc.tensor.matmul(
                                pss[n][:],
                                w_bf[k][:, 128 * m : 128 * (m + 1)],
                                xbcs[k][:, NTILE * n : NTILE * (n + 1)],
                                start=(k == 0),
                                stop=(k == NK - 1),
                            )
                    # Bias-add drains PSUM into one full-plane tile so the
                    # store has 12.5 KB contiguous runs per partition.
                    ot = out_pool.tile(
                        [128, PLANE], mybir.dt.float32, tag="ot", name=f"ot{b}_{m}"
                    )
                    for n in range(NN):
                        nc.vector.tensor_scalar_add(
                            ot[:, NTILE * n : NTILE * (n + 1)], pss[n][:], bias_t[m][:]
                        )
                    obase = (b * C + 128 * m) * PLANE
                    dst = o_ap[obase : obase + 128 * PLANE].rearrange(
                        "(p q) -> p q", q=PLANE
                    )
                    # Stores ride the Sync engine's HWDGE ring: store traffic
                    # never head-of-line-blocks the SWDGE load rings.
                    nc.sync.dma_start(dst, ot[:])

    nc.compile()
    return nc


def _get_program():
    global _COMPILED
    if _COMPILED is None:
        _COMPILED = _build_program()
    return _COMPILED


# Set by test harness to request an NTFF-profiled run; results stashed here.
TRACE = False
LAST_EXEC_TIME_NS = None


def pack_x(x_local):
    """Pack one core's (BL, C, H, W) slice into the padded channel-major
    flat layout the device program reads."""
    xi = np.zeros(NX_ELEMS, dtype=np.float32)
    view = xi[: C * CBLK].reshape(C, BL, H, WPAD)
    view[..., :W] = np.transpose(x_local, (1, 0, 2, 3))
    return xi


def kernel(x, weight, bias):
    global LAST_EXEC_TIME_NS
    x = np.ascontiguousarray(np.asarray(x, dtype=np.float32))
    weight = np.asarray(weight, dtype=np.float32)
    bias = np.ascontiguousarray(np.asarray(bias, dtype=np.float32))

    # Pure layout transform (no arithmetic): transpose + channel-permute the
    # weight so device partition p of contraction chunk k holds original
    # channel PERM[128k + p], matching the activation segment layout.
    wtp = np.ascontiguousarray(weight[:, PERM].T)

    nc = _get_program()

    in_maps = [
        {"x": pack_x(x[i * BL : (i + 1) * BL]), "wt": wtp, "bias": bias}
        for i in range(NCORES)
    ]

    res = run_bass_kernel_spmd(
        nc, in_maps, list(range(NCORES)), trace=TRACE
    )
    LAST_EXEC_TIME_NS = res.exec_time_ns

    out = np.empty((B, C, H, W), dtype=np.float32)
    for i in range(NCORES):
        out[i * BL : (i + 1) * BL] = res.results[i]["out"].reshape(BL, C, H, W)
    return out


# revision 20
# speedup vs baseline: 1.0524x; 1.0524x over previous
"""CycleFC (1-bit weights/activations) Trainium2 kernel.

Computes, for x (B=32, C=384, H=56, W=56), weight (C, C), bias (C,):
    xb = sign(x); wb = sign(weight)
    shifted[b,c,h,w] = xb[b,c,h,w+dx_c]  (0 outside [0,W)), dx_c = (c+3)%7-3
    out = einsum('bchw,oc->bohw', shifted, wb) + bias

Strategy (8 NeuronCores, SPMD):
  - Data-parallel over batch: 4 batches per core; weight/bias replicated.
  - The host stores each 56-wide row padded to 59 with zeros.  The
    per-channel horizontal shift then folds into the input DMA for free:
    for a fixed shift dx, the shifted plane is just the flat padded plane
    read at offset +dx -- positions that fall outside [0, W) pick up the
    row padding, which is exactly the required zero padding.
  - Channels are processed in a permuted order (grouped by c mod 7 ==
    constant shift) so each shift group is a partition-contiguous,
    channel-stride-7 affine DMA segment.  The weight matrix is permuted
    identically on the host (pure layout transform, no arithmetic), which
    leaves the GEMM result unchanged.
  - Loads are SWDGE (gpsimd) with an inline fp32->bf16 cast (+-1 is exact
    in bf16 and the 384-term accumulation is exact in fp32 PSUM, so the
    result is bit-identical to an fp32 computation).  Loads for 3 batches
    are kept in flight (software pipeline).
  - sign() runs on the Scalar engine reading the padded strided view and
    writing a compact [128, H*W] tile, so matmul rhs slices are contiguous.
  - GEMM: out[o, p] = sum_c wbT[c, o] * xb[c, p] on the Tensor engine,
    K = 384 contracted in 3 chunks of 128, k-outer over 7 live PSUM banks
    so the stationary weights are reused across pixel tiles.
  - Bias add fused into the PSUM -> SBUF drain on the Vector engine, into
    full-plane tiles so stores have 12.5 KB contiguous runs per partition;
    stores ride the Sync engine's HWDGE ring, separate from the SWDGE
    load rings.
"""

import numpy as np

import concourse.bass as bass
import concourse.tile as tile
from concourse import bacc, mybir
from concourse.bass_utils import run_bass_kernel_spmd

# Problem constants (hardcoded per spec)
B, C, H, W = 32, 384, 56, 56
PLANE = H * W              # 3136 (unpadded output plane)
NCORES = 8
BL = B // NCORES           # 4 batches per core
KS = 7                     # cyclic shift period (kernel_size 7)
NK = C // 128              # 3 contraction chunks
NM = C // 128              # 3 output-channel chunks
ROWS_PER_TILE = 8
NTILE = ROWS_PER_TILE * W  # 448 pixels per PSUM tile
NN = H // ROWS_PER_TILE    # 7 pixel tiles per (b, m)
WPAD = 59                  # row pitch: 56 data + 3 zero cols (>= max |dx|)
PLANE_P = H * WPAD         # 3304 (padded input plane)
BACK_PAD = 7 * PLANE_P     # slack so segment APs can over-claim past the end
NX_ELEMS = BL * C * PLANE_P + BACK_PAD
NOUT_ELEMS = BL * C * PLANE

# Shift-group segments of the permuted channel order.  perm = channels
# grouped by r = c mod 7 (r ascending, then c ascending within the group).
# Each segment is a partition-contiguous run inside one 128-channel chunk:
# (chunk, part_start, nseg, c_first, dx) with original channels
# c_first + 7*i for i in [0, nseg).
SEGMENTS = [
    (0, 0, 55, 0, 0),
    (0, 55, 55, 1, 1),
    (0, 110, 18, 2, 2),
    (1, 0, 37, 128, 2),    # r=2 continued: 2 + 7*18
    (1, 37, 55, 3, 3),
    (1, 92, 36, 4, -3),
    (2, 0, 19, 256, -3),   # r=4 continued: 4 + 7*36
    (2, 19, 55, 5, -2),
    (2, 74, 54, 6, -1),
]

PERM = np.concatenate([np.arange(r, C, KS) for r in range(KS)])

_COMPILED = None


def _build_program():
    """Trace + compile the single-core Bass program (same on all 8 cores)."""
    nc = bacc.Bacc(
        "TRN2",
        target_bir_lowering=False,
        debug=False,
        num_devices=NCORES,
    )
    x_d = nc.dram_tensor("x", [NX_ELEMS], mybir.dt.float32, kind="ExternalInput")
    w_d = nc.dram_tensor("wt", [C, C], mybir.dt.float32, kind="ExternalInput")
    b_d = nc.dram_tensor("bias", [C], mybir.dt.float32, kind="ExternalInput")
    o_d = nc.dram_tensor("out", [NOUT_ELEMS], mybir.dt.float32, kind="ExternalOutput")

    x_ap = x_d.ap()
    o_ap = o_d.ap()

    segs_by_chunk = [[s[1:] for s in SEGMENTS if s[0] == k] for k in range(NK)]

    with tile.TileContext(nc) as tc:
        with (
            tc.tile_pool(name="const", bufs=1) as cpool,
            tc.tile_pool(name="xbr", bufs=9) as xbr_pool,
            tc.tile_pool(name="xbc", bufs=9) as xbc_pool,
            tc.tile_pool(name="psum", bufs=8, space="PSUM") as psum_pool,
            tc.tile_pool(name="outs", bufs=4) as out_pool,
        ):
            # Weights/bias first on the SWDGE ring so they complete before
            # the big x loads contend for the SDMA engines.
            wraws = []
            for k in range(NK):
                wraw = cpool.tile([128, C], mybir.dt.float32, tag=f"wraw{k}")
                nc.gpsimd.dma_start(wraw[:], w_d.ap()[128 * k : 128 * (k + 1), :])
                wraws.append(wraw)
            bias_t = []
            for m in range(NM):
                bt = cpool.tile([128, 1], mybir.dt.float32, tag=f"bias{m}")
                nc.gpsimd.dma_start(bt[:], b_d.ap()[128 * m : 128 * (m + 1)].unsqueeze(1))
                bias_t.append(bt)
            # Binarized, pre-transposed, channel-permuted weights: wbT[c, o].
            w_bf = []
            for k in range(NK):
                wb = cpool.tile([128, C], mybir.dt.bfloat16, tag=f"wb{k}")
                nc.scalar.sign(wb[:], wraws[k][:])
                w_bf.append(wb)

            xbrs = {}

            def emit_loads(b):
                # SWDGE loads with inline fp32->bf16 cast (sign-preserving).
                tiles = []
                for k in range(NK):
                    xbr = xbr_pool.tile(
                        [128, PLANE_P], mybir.dt.bfloat16, tag="xbr", name=f"xbr{b}_{k}"
                    )
                    for (part_start, nseg, c_first, dx) in segs_by_chunk[k]:
                        base = (b * C + c_first) * PLANE_P + dx
                        src = (
                            x_ap[base : base + nseg * KS * PLANE_P]
                            .rearrange("(p q) -> p q", q=KS * PLANE_P)[:, :PLANE_P]
                        )
                        nc.gpsimd.dma_start(xbr[part_start : part_start + nseg, :], src)
                    tiles.append(xbr)
                xbrs[b] = tiles

            # Software pipeline: keep 3 batches of loads in flight so the
            # Scalar/Tensor engines never starve between batch iterations.
            emit_loads(0)
            emit_loads(1)
            emit_loads(2)

            # Sign is split at an n-tile boundary (rows 0-23 / 24-55) so the
            # first matmuls of each k-row unblock after half the binarize.
            HSPLIT = 3 * ROWS_PER_TILE  # 24 rows

            for b in range(BL):
                xbcs = []
                for k in range(NK):
                    # Binarize + drop the pad columns: strided read of the
                    # [H, :W] view, contiguous [128, H*W] write.
                    xbc = xbc_pool.tile(
                        [128, PLANE], mybir.dt.bfloat16, tag="xbc", name=f"xbc{b}_{k}"
                    )
                    dstv = xbc[:].rearrange("p (h w) -> p h w", w=W)
                    srcv = xbrs[b][k][:].rearrange("p (h w) -> p h w", w=WPAD)[:, :, :W]
                    nc.scalar.sign(dstv[:, :HSPLIT, :], srcv[:, :HSPLIT, :])
                    nc.scalar.sign(dstv[:, HSPLIT:, :], srcv[:, HSPLIT:, :])
                    xbcs.append(xbc)
                del xbrs[b]

                for m in range(NM):
                    pss = [
                        psum_pool.tile(
                            [128, NTILE], mybir.dt.float32, tag="ps", name=f"ps{b}_{m}_{n}"
                        )
                        for n in range(NN)
                    ]
                    # k-outer: the stationary weight chunk is reused across
                    # the 7 pixel tiles; PSUM accumulates across k.
                    for k in range(NK):
                        for n in range(NN):
                            nc.tensor.matmul(
                                pss[n][:],
                                w_bf[k][:, 128 * m : 128 * (m + 1)],
                                xbcs[k][:, NTILE * n : NTILE * (n + 1)],
                                start=(k == 0),
                                stop=(k == NK - 1),
                            )
                    # Bias-add drains PSUM into one full-plane tile so the
                    # store has 12.5 KB contiguous runs per partition.
                    ot = out_pool.tile(
                        [128, PLANE], mybir.dt.float32, tag="ot", name=f"ot{b}_{m}"
                    )
                    for n in range(NN):
                        nc.vector.tensor_scalar_add(
                            ot[:, NTILE * n : NTILE * (n + 1)], pss[n][:], bias_t[m][:]
                        )
                    obase = (b * C + 128 * m) * PLANE
                    dst = o_ap[obase : obase + 128 * PLANE].rearrange(
                        "(p q) -> p q", q=PLANE
                    )
                    # Stores ride the Sync engine's HWDGE ring: store traffic
                    # never head-of-line-blocks the SWDGE load rings.
                    nc.sync.dma_start(dst, ot[:])

                if b + 3 < BL:
                    emit_loads(b + 3)

    nc.compile()
    return nc


def _get_program():
    global _COMPILED
    if _COMPILED is None:
        _COMPILED = _build_program()
    return _COMPILED


# Set by test harness to request an NTFF-profiled run; results stashed here.
TRACE = False
LAST_EXEC_TIME_NS = None


def pack_x(x_local):
    """Pack one core's (BL, C, H, W) slice into the padded flat layout the
    device program reads."""
    xi = np.zeros(NX_ELEMS, dtype=np.float32)
    view = xi[: BL * C * PLANE_P].reshape(BL, C, H, WPAD)
    view[..., :W] = x_local
    return xi


def kernel(x, weight, bias):
    global LAST_EXEC_TIME_NS
    x = np.ascontiguousarray(np.asarray(x, dtype=np.float32))
    weight = np.asarray(weight, dtype=np.float32)
    bias = np.ascontiguousarray(np.asarray(bias, dtype=np.float32))

    # Pure layout transform (no arithmetic): transpose + channel-permute the
    # weight so device partition p of contraction chunk k holds original
    # channel PERM[128k + p], matching the activation segment layout.
    wtp = np.ascontiguousarray(weight[:, PERM].T)

    nc = _get_program()

    in_maps = [
        {"x": pack_x(x[i * BL : (i + 1) * BL]), "wt": wtp, "bias": bias}
        for i in range(NCORES)
    ]

    res = run_bass_kernel_spmd(
        nc, in_maps, list(range(NCORES)), trace=TRACE
    )
    LAST_EXEC_TIME_NS = res.exec_time_ns

    out = np.empty((B, C, H, W), dtype=np.float32)
    for i in range(NCORES):
        out[i * BL : (i + 1) * BL] = res.results[i]["out"].reshape(BL, C, H, W)
    return out


# revision 21
# speedup vs baseline: 1.1258x; 1.0698x over previous
"""CycleFC (1-bit weights/activations) Trainium2 kernel.

Computes, for x (B=32, C=384, H=56, W=56), weight (C, C), bias (C,):
    xb = sign(x); wb = sign(weight)
    shifted[b,c,h,w] = xb[b,c,h,w+dx_c]  (0 outside [0,W)), dx_c = (c+3)%7-3
    out = einsum('bchw,oc->bohw', shifted, wb) + bias

Strategy (8 NeuronCores, SPMD):
  - Data-parallel over batch: 4 batches per core; weight/bias replicated.
  - The host stores each 56-wide row padded to 59 with zeros.  The
    per-channel horizontal shift then folds into the input DMA for free:
    for a fixed shift dx, the shifted plane is just the flat padded plane
    read at offset +dx -- positions that fall outside [0, W) pick up the
    row padding, which is exactly the required zero padding.
  - Channels are processed in a permuted order (grouped by c mod 7 ==
    constant shift) so each shift group is a partition-contiguous,
    channel-stride-7 affine DMA segment.  The weight matrix is permuted
    identically on the host (pure layout transform, no arithmetic), which
    leaves the GEMM result unchanged.
  - Loads are SWDGE (gpsimd) with an inline fp32->bf16 cast (+-1 is exact
    in bf16 and the 384-term accumulation is exact in fp32 PSUM, so the
    result is bit-identical to an fp32 computation).  Loads for 3 batches
    are kept in flight (software pipeline).
  - sign() runs on the Scalar engine reading the padded strided view and
    writing a compact [128, H*W] tile, so matmul rhs slices are contiguous.
  - GEMM: out[o, p] = sum_c wbT[c, o] * xb[c, p] on the Tensor engine,
    K = 384 contracted in 3 chunks of 128, k-outer over 7 live PSUM banks
    so the stationary weights are reused across pixel tiles.
  - Bias add fused into the PSUM -> SBUF drain on the Vector engine, into
    full-plane tiles so stores have 12.5 KB contiguous runs per partition;
    stores ride the Sync engine's HWDGE ring, separate from the SWDGE
    load rings.
"""

import numpy as np

import concourse.bass as bass
import concourse.tile as tile
from concourse import bacc, mybir
from concourse.bass_utils import run_bass_kernel_spmd

# Problem constants (hardcoded per spec)
B, C, H, W = 32, 384, 56, 56
PLANE = H * W              # 3136 (unpadded output plane)
NCORES = 8
BL = B // NCORES           # 4 batches per core
KS = 7                     # cyclic shift period (kernel_size 7)
NK = C // 128              # 3 contraction chunks
NM = C // 128              # 3 output-channel chunks
ROWS_PER_TILE = 8
NTILE = ROWS_PER_TILE * W  # 448 pixels per PSUM tile
NN = H // ROWS_PER_TILE    # 7 pixel tiles per (b, m)
WPAD = 59                  # row pitch: 56 data + 3 zero cols (>= max |dx|)
PLANE_P = H * WPAD         # 3304 (padded input plane)
BACK_PAD = 7 * PLANE_P     # slack so segment APs can over-claim past the end
NX_ELEMS = BL * C * PLANE_P + BACK_PAD
NOUT_ELEMS = BL * C * PLANE

# Shift-group segments of the permuted channel order.  perm = channels
# grouped by r = c mod 7 (r ascending, then c ascending within the group).
# Each segment is a partition-contiguous run inside one 128-channel chunk:
# (chunk, part_start, nseg, c_first, dx) with original channels
# c_first + 7*i for i in [0, nseg).
SEGMENTS = [
    (0, 0, 55, 0, 0),
    (0, 55, 55, 1, 1),
    (0, 110, 18, 2, 2),
    (1, 0, 37, 128, 2),    # r=2 continued: 2 + 7*18
    (1, 37, 55, 3, 3),
    (1, 92, 36, 4, -3),
    (2, 0, 19, 256, -3),   # r=4 continued: 4 + 7*36
    (2, 19, 55, 5, -2),
    (2, 74, 54, 6, -1),
]

PERM = np.concatenate([np.arange(r, C, KS) for r in range(KS)])

_COMPILED = None


def _build_program():
    """Trace + compile the single-core Bass program (same on all 8 cores)."""
    nc = bacc.Bacc(
        "TRN2",
        target_bir_lowering=False,
        debug=False,
        num_devices=NCORES,
    )
    x_d = nc.dram_tensor("x", [NX_ELEMS], mybir.dt.float32, kind="ExternalInput")
    w_d = nc.dram_tensor("wt", [C, C], mybir.dt.float32, kind="ExternalInput")
    b_d = nc.dram_tensor("bias", [C], mybir.dt.float32, kind="ExternalInput")
    o_d = nc.dram_tensor("out", [NOUT_ELEMS], mybir.dt.float32, kind="ExternalOutput")

    x_ap = x_d.ap()
    o_ap = o_d.ap()

    segs_by_chunk = [[s[1:] for s in SEGMENTS if s[0] == k] for k in range(NK)]

    with tile.TileContext(nc) as tc:
        with (
            tc.tile_pool(name="const", bufs=1) as cpool,
            tc.tile_pool(name="xbr", bufs=9) as xbr_pool,
            tc.tile_pool(name="xbc", bufs=9) as xbc_pool,
            tc.tile_pool(name="psum", bufs=8, space="PSUM") as psum_pool,
            tc.tile_pool(name="outs", bufs=4) as out_pool,
        ):
            # Weights/bias first on the SWDGE ring so they complete before
            # the big x loads contend for the SDMA engines.
            wraws = []
            for k in range(NK):
                wraw = cpool.tile([128, C], mybir.dt.float32, tag=f"wraw{k}")
                nc.gpsimd.dma_start(wraw[:], w_d.ap()[128 * k : 128 * (k + 1), :])
                wraws.append(wraw)
            bias_t = []
            for m in range(NM):
                bt = cpool.tile([128, 1], mybir.dt.float32, tag=f"bias{m}")
                nc.gpsimd.dma_start(bt[:], b_d.ap()[128 * m : 128 * (m + 1)].unsqueeze(1))
                bias_t.append(bt)
            # Binarized, pre-transposed, channel-permuted weights: wbT[c, o].
            w_bf = []
            for k in range(NK):
                wb = cpool.tile([128, C], mybir.dt.bfloat16, tag=f"wb{k}")
                nc.scalar.sign(wb[:], wraws[k][:])
                w_bf.append(wb)

            xbrs = {}

            def emit_loads(b):
                # SWDGE loads with inline fp32->bf16 cast (sign-preserving).
                tiles = []
                for k in range(NK):
                    xbr = xbr_pool.tile(
                        [128, PLANE_P], mybir.dt.bfloat16, tag="xbr", name=f"xbr{b}_{k}"
                    )
                    for (part_start, nseg, c_first, dx) in segs_by_chunk[k]:
                        base = (b * C + c_first) * PLANE_P + dx
                        src = (
                            x_ap[base : base + nseg * KS * PLANE_P]
                            .rearrange("(p q) -> p q", q=KS * PLANE_P)[:, :PLANE_P]
                        )
                        nc.gpsimd.dma_start(xbr[part_start : part_start + nseg, :], src)
                    tiles.append(xbr)
                xbrs[b] = tiles

            # Software pipeline: keep 3 batches of loads in flight so the
            # Scalar/Tensor engines never starve between batch iterations.
            emit_loads(0)
            emit_loads(1)
            emit_loads(2)

            # Sign is split at an n-tile boundary (rows 0-23 / 24-55) so the
            # first matmuls of each k-row unblock after half the binarize.
            HSPLIT = 3 * ROWS_PER_TILE  # 24 rows

            for b in range(BL):
                xbcs = []
                for k in range(NK):
                    # Binarize + drop the pad columns: strided read of the
                    # [H, :W] view, contiguous [128, H*W] write.
                    xbc = xbc_pool.tile(
                        [128, PLANE], mybir.dt.bfloat16, tag="xbc", name=f"xbc{b}_{k}"
                    )
                    dstv = xbc[:].rearrange("p (h w) -> p h w", w=W)
                    srcv = xbrs[b][k][:].rearrange("p (h w) -> p h w", w=WPAD)[:, :, :W]
                    nc.scalar.sign(dstv[:, :HSPLIT, :], srcv[:, :HSPLIT, :])
                    nc.scalar.sign(dstv[:, HSPLIT:, :], srcv[:, HSPLIT:, :])
                    xbcs.append(xbc)
                del xbrs[b]

                for m in range(NM):
                    pss = [
                        psum_pool.tile(
                            [128, NTILE], mybir.dt.float32, tag="ps", name=f"ps{b}_{m}_{n}"
                        )
                        for n in range(NN)
                    ]
                    # k-outer: the stationary weight chunk is reused across
                    # the 7 pixel tiles; PSUM accumulates across k.
                    for k in range(NK):
                        for n in range(NN):
                            nc.tensor.matmul(
                                pss[n][:],
                                w_bf[k][:, 128 * m : 128 * (m + 1)],
                                xbcs[k][:, NTILE * n : NTILE * (n + 1)],
                                start=(k == 0),
                                stop=(k == NK - 1),
                            )
                    # Bias-add drains PSUM into one full-plane tile; the
                    # store is split in two (rows 0-23 / 24-55, both still
                    # multi-KB contiguous runs per partition) so the first
                    # half streams out after 3 of 7 bias-adds instead of
                    # bursting the whole plane at the end.
                    ot = out_pool.tile(
                        [128, PLANE], mybir.dt.float32, tag="ot", name=f"ot{b}_{m}"
                    )
                    obase = (b * C + 128 * m) * PLANE
                    dst = o_ap[obase : obase + 128 * PLANE].rearrange(
                        "(p q) -> p q", q=PLANE
                    )
                    SPLIT = HSPLIT * W  # 1344 px = n-tiles 0..2
                    for n in range(NN):
                        nc.vector.tensor_scalar_add(
                            ot[:, NTILE * n : NTILE * (n + 1)], pss[n][:], bias_t[m][:]
                        )
                        # Stores ride the Sync engine's HWDGE ring: store
                        # traffic never blocks the SWDGE load rings.
                        if n == 2:
                            nc.sync.dma_start(dst[:, :SPLIT], ot[:, :SPLIT])
                        elif n == NN - 1:
                            nc.sync.dma_start(dst[:, SPLIT:], ot[:, SPLIT:])

                if b + 3 < BL:
                    emit_loads(b + 3)

    nc.compile()
    return nc


def _get_program():
    global _COMPILED
    if _COMPILED is None:
        _COMPILED = _build_program()
    return _COMPILED


# Set by test harness to request an NTFF-profiled run; results stashed here.
TRACE = False
LAST_EXEC_TIME_NS = None


def pack_x(x_local):
    """Pack one core's (BL, C, H, W) slice into the padded flat layout the
    device program reads."""
    xi = np.zeros(NX_ELEMS, dtype=np.float32)
    view = xi[: BL * C * PLANE_P].reshape(BL, C, H, WPAD)
    view[..., :W] = x_local
    return xi


def kernel(x, weight, bias):
    global LAST_EXEC_TIME_NS
    x = np.ascontiguousarray(np.asarray(x, dtype=np.float32))
    weight = np.asarray(weight, dtype=np.float32)
    bias = np.ascontiguousarray(np.asarray(bias, dtype=np.float32))

    # Pure layout transform (no arithmetic): transpose + channel-permute the
    # weight so device partition p of contraction chunk k holds original
    # channel PERM[128k + p], matching the activation segment layout.
    wtp = np.ascontiguousarray(weight[:, PERM].T)

    nc = _get_program()

    in_maps = [
        {"x": pack_x(x[i * BL : (i + 1) * BL]), "wt": wtp, "bias": bias}
        for i in range(NCORES)
    ]

    res = run_bass_kernel_spmd(
        nc, in_maps, list(range(NCORES)), trace=TRACE
    )
    LAST_EXEC_TIME_NS = res.exec_time_ns

    out = np.empty((B, C, H, W), dtype=np.float32)
    for i in range(NCORES):
        out[i * BL : (i + 1) * BL] = res.results[i]["out"].reshape(BL, C, H, W)
    return out
